# revision 17
# baseline (speedup 1.0000x reference)
"""BlockSparseAttention Trainium2 kernel (8 NeuronCores, SPMD).

Problem: B=4, S=4096, E=1024, H=16 heads, D=64, block=128, window = self +/- 1 block.
  Q/K/V projections -> block-local + windowed-cross attention -> output projection.

Sharding: core c = (batch b=c//2, seq half hf=c%2). Each core owns 16 q-blocks
(2048 rows) and a 17-block K/V slab (own 16 blocks + 1 halo block). The second
half is BLOCK-REVERSED on host so that the pad/halo structure is identical on
every core (uniform SPMD program):
  chunk 0 of the conceptual 18-chunk slab is always the invalid pad block and is
  simply skipped; shipped slab = chunks 1..17 (2176 rows).
  q-block i attends slab chunks {i, i+1, i+2}; self block = chunk i+1.

On-chip dataflow (all matmuls bf16 w/ fp32 PSUM accumulation):
  Q^T[e,s] = WqT-chunk.T @ qT      (lhsT=WqT [ein,eo], rhs=qT [ein,s])
  K^T[e,s] likewise -> bounced through DRAM, reloaded per head-pair
  V[s,e]   = vT-chunk.T @ WvT      (natural layout, + ones column for denoms)
  S^T[kk,q] = (K^T chunk).T @ Q^T  per (head, chunk) -> packed PSUM
  expS = exp(0.125 * S^T)          ACT, PSUM->SBUF bf16, packed [128, 6016]
  PV: out[q, 0:65] = expS-chunk.T @ [ones|V]  (col 0 = softmax denominator)
      cross accumulates 2-3 chunks; local = separate single-chunk group
  normalize per-partition (q on partitions), PE-transpose per block pair,
  collect attn^T[e, s] bf16, then O-projection + bias.
"""

import math
from contextlib import ExitStack

import numpy as np
import ml_dtypes

import concourse.bacc as bacc
import concourse.bass as bass
import concourse.mybir as mybir
import concourse.tile as tile
from concourse import bass_utils
from concourse.masks import make_identity

F32 = mybir.dt.float32
BF16 = mybir.dt.bfloat16

E = 1024
H = 16
D = 64
BS = 128
B = 4
S = 4096
NBQ = 16           # q blocks per core
SQ = NBQ * BS      # 2048
NCH = 17           # shipped kv chunks (c = 1..17)
SKV = NCH * BS     # 2176
NPAIR = 8          # head pairs
KCH = 8            # e_in chunks of 128
SCALE = 1.0 / math.sqrt(D)

_BF = ml_dtypes.bfloat16


def _chunk_qb(c):
    """Valid q-block range [qb0, qb1] attending slab chunk c (1..17)."""
    return max(0, c - 2), min(NBQ - 1, c)


def _score_layout():
    """Packed column layout of the per-head score matrix exp(S^T).

    Returns (chunk_base dict, total_cols). Chunk c occupies packed cols
    [chunk_base[c], chunk_base[c] + width_c) where width_c = 128 * n_valid_qblocks.
    """
    base = 0
    chunk_base = {}
    for c in range(1, NCH + 1):
        qb0, qb1 = _chunk_qb(c)
        chunk_base[c] = base
        base += (qb1 - qb0 + 1) * BS
    return chunk_base, base


CHUNK_BASE, SCORE_COLS = _score_layout()   # SCORE_COLS == 6016
SCORE_TILE = 1024                          # score psum tile width (2 banks)
N_SCORE_TILES = (SCORE_COLS + SCORE_TILE - 1) // SCORE_TILE


def _build():
    nc = bacc.Bacc(None, target_bir_lowering=False)

    qT_t = nc.dram_tensor("qT", [E, SQ], BF16, kind="ExternalInput")
    kT_t = nc.dram_tensor("kT", [E, SKV], BF16, kind="ExternalInput")
    vT_t = nc.dram_tensor("vT", [E, SKV], BF16, kind="ExternalInput")
    wqT_t = nc.dram_tensor("wqT", [E, E], BF16, kind="ExternalInput")
    wkT_t = nc.dram_tensor("wkT", [E, E], BF16, kind="ExternalInput")
    wvT_t = nc.dram_tensor("wvT", [E, E], BF16, kind="ExternalInput")
    woT_t = nc.dram_tensor("woT", [E, E], BF16, kind="ExternalInput")
    bq_t = nc.dram_tensor("bq2", [128, 8], F32, kind="ExternalInput")
    bk_t = nc.dram_tensor("bk2", [128, 8], F32, kind="ExternalInput")
    bvb_t = nc.dram_tensor("bvb", [128, E], F32, kind="ExternalInput")
    bob_t = nc.dram_tensor("bob", [128, E], F32, kind="ExternalInput")
    out_t = nc.dram_tensor("out", [SQ, E], F32, kind="ExternalOutput")
    dbg_qt = nc.dram_tensor("dbg_qt", [128, 8, SQ], BF16, kind="ExternalOutput")
    dbg_kt = nc.dram_tensor("dbg_kt", [128, SKV], BF16, kind="ExternalOutput")
    dbg_v = nc.dram_tensor("dbg_v", [128, H * NCH * 65], BF16, kind="ExternalOutput")
    dbg_es = nc.dram_tensor("dbg_es", [128, SCORE_COLS], BF16, kind="ExternalOutput")
    dbg_at = nc.dram_tensor("dbg_at", [128, 8, SQ], BF16, kind="ExternalOutput")
    dbg_pv = nc.dram_tensor("dbg_pv", [128, NBQ * 65], F32, kind="ExternalOutput")
    dbg_an = nc.dram_tensor("dbg_an", [128, NBQ * 64], BF16, kind="ExternalOutput")



    qT_d = qT_t[:].rearrange("(a p) s -> p a s", p=128)       # [128, 8, 2048]
    kT_d = kT_t[:].rearrange("(a p) s -> p a s", p=128)       # [128, 8, 2176]
    vT_d = vT_t[:].rearrange("(a p) s -> p a s", p=128)
    wq_d = wqT_t[:].rearrange("(a p) f -> p a f", p=128)      # [128, 8, 1024]
    wk_d = wkT_t[:].rearrange("(a p) f -> p a f", p=128)
    wv_d = wvT_t[:].rearrange("(a p) f -> p a f", p=128)
    wo_d = woT_t[:].rearrange("(a p) f -> p a f", p=128)

    with TileCtx(nc) as tc:
        with (
            tc.tile_pool(name="dram", bufs=1, space="DRAM") as dpool,
            tc.tile_pool(name="const", bufs=1) as cpool,
            tc.tile_pool(name="big", bufs=1) as big,
            tc.tile_pool(name="wpool", bufs=1) as wpool,
            tc.tile_pool(name="stage", bufs=2) as stage,
            tc.tile_pool(name="ktpair", bufs=2) as ktpool,
            tc.tile_pool(name="exps", bufs=2) as epool,
            tc.tile_pool(name="small", bufs=4) as small,
            tc.tile_pool(name="outp", bufs=2) as outp,
        ):
            kt_dram = dpool.tile([E, SKV], BF16, name="kt_bounce")
            ktb_d = kt_dram.rearrange("(a p) s -> p a s", p=128)  # [128,8,2176]

            ident = cpool.tile([128, 128], BF16, name="ident")
            make_identity(nc, ident)
            bq_sb = cpool.tile([128, 8], F32, name="bq_sb")
            nc.sync.dma_start(bq_sb, bq_t[:, :])
            bk_sb = cpool.tile([128, 8], F32, name="bk_sb")
            nc.sync.dma_start(bk_sb, bk_t[:, :])
            bvb_sb = cpool.tile([128, E], F32, name="bvb_sb")
            nc.sync.dma_start(bvb_sb, bvb_t[:, :])
            bob_sb = cpool.tile([128, E], F32, name="bob_sb")
            nc.sync.dma_start(bob_sb, bob_t[:, :])

            # persistent big tensors
            QT_sb = big.tile([128, 8, SQ], BF16, name="QT_sb")       # Q^T
            V_all = big.tile([128, H, NCH, 65], BF16, name="V_all")  # [ones|V]
            V_flat = V_all.rearrange("p h c d -> p (h c d)")
            attnT = big.tile([128, 8, SQ], BF16, name="attnT")       # attn^T

            nc.gpsimd.memset(V_all[:, :, :, 0:1], 1.0)
            # DVE-local copy of the V bias so the V scatter copies depend on
            # DVE work only (instruction wait-slot limits on TensorTensor).
            bvb2 = cpool.tile([128, E], F32, name="bvb2")
            nc.vector.tensor_copy(out=bvb2, in_=bvb_sb)
            bob2 = cpool.tile([128, E], F32, name="bob2")
            nc.vector.tensor_copy(out=bob2, in_=bob_sb)
            bq2d = cpool.tile([128, 8], F32, name="bq2d")
            nc.vector.tensor_copy(out=bq2d, in_=bq_sb)
            bk2d = cpool.tile([128, 8], F32, name="bk2d")
            nc.vector.tensor_copy(out=bk2d, in_=bk_sb)

            # ---------------- projection phase ----------------
            with tc.tile_pool(name="ppsum", bufs=3, space="PSUM") as ppsum:
                # V projection: V[s, e_out] = vT_chunk.T @ WvT
                wv_sb = wpool.tile([128, 8, E], BF16, name="wv_sb", tag="w")
                nc.sync.dma_start(wv_sb, wv_d)
                for sg in range(5):                      # 4x512 + 1x128 cols of vT
                    w_sg = min(512, SKV - 512 * sg)
                    vstage = stage.tile([128, 8, 512], BF16, name="vstage",
                                        tag="xstage")
                    nc.sync.dma_start(vstage[:, :, :w_sg],
                                      vT_d[:, :, 512 * sg:512 * sg + w_sg])
                    for sc4 in range(w_sg // 128):       # s-chunks of 128
                        sc = 4 * sg + sc4
                        for eh in range(2):              # e_out halves of 512
                            psum = ppsum.tile([128, 512], F32, name="vpsum",
                                              tag="ppsum")
                            for kk in range(KCH):
                                nc.tensor.matmul(
                                    psum,
                                    lhsT=vstage[:, kk, 128 * sc4:128 * sc4 + 128],
                                    rhs=wv_sb[:, kk, 512 * eh:512 * eh + 512],
                                    start=(kk == 0), stop=(kk == KCH - 1),
                                )
                            # add bias, cast bf16, scatter to per-head layout
                            # (per-head 2D copies: a 3D+3D+3D tensor_tensor
                            # overflows the TT instruction's sync-wait slots)
                            for j in range(8):
                                h = 8 * eh + j
                                off = (h * NCH + sc) * 65 + 1
                                nc.vector.tensor_tensor(
                                    V_flat[:, off:off + 64],
                                    psum[:, 64 * j:64 * j + 64],
                                    bvb2[:, 64 * h:64 * h + 64],
                                    mybir.AluOpType.add,
                                )

                # Q projection: Q^T[e_out, s] = WqT_chunk.T @ qT
                wq_sb = wpool.tile([128, 8, E], BF16, name="wq_sb", tag="w")
                nc.sync.dma_start(wq_sb, wq_d)
                for st in range(4):                      # s tiles of 512
                    qstage = stage.tile([128, 8, 512], BF16, name="qstage",
                                        tag="xstage")
                    nc.sync.dma_start(qstage,
                                      qT_d[:, :, 512 * st:512 * st + 512])
                    for pr in range(NPAIR):
                        psum = ppsum.tile([128, 512], F32, name="qpsum",
                                          tag="ppsum")
                        for kk in range(KCH):
                            nc.tensor.matmul(
                                psum,
                                lhsT=wq_sb[:, kk, 128 * pr:128 * pr + 128],
                                rhs=qstage[:, kk, :],
                                start=(kk == 0), stop=(kk == KCH - 1),
                            )
                        nc.vector.tensor_scalar(
                            QT_sb[:, pr, 512 * st:512 * st + 512],
                            psum, bq2d[:, pr:pr + 1], None,
                            mybir.AluOpType.add,
                        )

                # K projection -> DRAM bounce (per head-pair rows)
                wk_sb = wpool.tile([128, 8, E], BF16, name="wk_sb", tag="w")
                nc.sync.dma_start(wk_sb, wk_d)
                for st in range(5):
                    w_st = min(512, SKV - 512 * st)
                    kstage = stage.tile([128, 8, 512], BF16, name="kstage",
                                        tag="xstage")
                    nc.sync.dma_start(kstage[:, :, :w_st],
                                      kT_d[:, :, 512 * st:512 * st + w_st])
                    for pr in range(NPAIR):
                        psum = ppsum.tile([128, 512], F32, name="kpsum",
                                          tag="ppsum")
                        for kk in range(KCH):
                            nc.tensor.matmul(
                                psum[:, :w_st],
                                lhsT=wk_sb[:, kk, 128 * pr:128 * pr + 128],
                                rhs=kstage[:, kk, :w_st],
                                start=(kk == 0), stop=(kk == KCH - 1),
                            )
                        ktmp = small.tile([128, 512], BF16, name="ktmp",
                                          tag="ktmp")
                        nc.vector.tensor_scalar(
                            ktmp[:, :w_st], psum[:, :w_st],
                            bk2d[:, pr:pr + 1], None, mybir.AluOpType.add,
                        )
                        nc.sync.dma_start(
                            ktb_d[:, pr, 512 * st:512 * st + w_st],
                            ktmp[:, :w_st])

            nc.sync.dma_start(dbg_qt[:, :, :], QT_sb)
            nc.sync.dma_start(dbg_v[:, :], V_flat)

            # ---------------- attention phase ----------------
            with (
                tc.tile_pool(name="spsum", bufs=2, space="PSUM") as spsum,
                tc.tile_pool(name="pvpsum", bufs=3, space="PSUM") as pvpsum,
                tc.tile_pool(name="tpsum", bufs=1, space="PSUM") as tpsum,
            ):
                for hp in range(NPAIR):
                    kt_pair = ktpool.tile([128, SKV], BF16, name="kt_pair",
                                          tag="ktpair")
                    nc.sync.dma_start(kt_pair, ktb_d[:, hp, :])
                    if hp == 0:
                        nc.sync.dma_start(dbg_kt[:, :], kt_pair)
                    for hh in range(2):
                        h = 2 * hp + hh
                        p0 = 64 * hh
                        expS = epool.tile([128, SCORE_COLS], BF16,
                                          name="expS", tag="expS")

                        # scores S^T, packed into [128, 1024] psum tiles
                        score_ps = {}
                        for c in range(1, NCH + 1):
                            qb0, qb1 = _chunk_qb(c)
                            cb = CHUNK_BASE[c]
                            width = (qb1 - qb0 + 1) * BS
                            pos = cb
                            while pos < cb + width:
                                # split at 512-boundaries of packed layout
                                nxt = min(cb + width, (pos // 512 + 1) * 512)
                                t = pos // SCORE_TILE
                                if t not in score_ps:
                                    score_ps[t] = spsum.tile(
                                        [128, SCORE_TILE], F32,
                                        name="score_ps", tag="spsum")
                                qcol = qb0 * BS + (pos - cb)
                                nc.tensor.matmul(
                                    score_ps[t][:, pos - SCORE_TILE * t:
                                                nxt - SCORE_TILE * t],
                                    lhsT=kt_pair[p0:p0 + 64,
                                                 128 * (c - 1):128 * c],
                                    rhs=QT_sb[p0:p0 + 64, hp,
                                              qcol:qcol + (nxt - pos)],
                                    start=True, stop=True,
                                )
                                pos = nxt
                                # exp as soon as a tile is complete
                                done = (c == NCH and pos == cb + width)
                                if pos % SCORE_TILE == 0 or done:
                                    tt = (pos - 1) // SCORE_TILE
                                    if tt in score_ps:
                                        wt = min(SCORE_TILE,
                                                 SCORE_COLS - SCORE_TILE * tt)
                                        nc.scalar.activation(
                                            expS[:, SCORE_TILE * tt:
                                                 SCORE_TILE * tt + wt],
                                            score_ps[tt][:, :wt],
                                            mybir.ActivationFunctionType.Exp,
                                            scale=SCALE,
                                        )
                                        del score_ps[tt]

                        if h == 0:
                            nc.sync.dma_start(dbg_es[:, :], expS)

                        # PV + normalize + transpose, per q-block.
                        # Single accumulation group per PSUM bank: the self
                        # chunk goes first and its partial state (= the local
                        # attention term) is snapshotted by DVE mid-group.
                        an = None
                        for i in range(NBQ):
                            chs = [i + 1] + [c for c in (i, i + 2) if c >= 1]
                            pv = pvpsum.tile([128, 65], F32, name="pv",
                                             tag="pv")
                            rec = small.tile([128, 2], F32, name="rec",
                                             tag="rec")
                            t1 = small.tile([128, 64], F32, name="t1", tag="t1")
                            t2 = small.tile([128, 64], F32, name="t2", tag="t2")
                            for j, c in enumerate(chs):
                                qb0, _ = _chunk_qb(c)
                                lcol = CHUNK_BASE[c] + (i - qb0) * BS
                                nc.tensor.matmul(
                                    pv,
                                    lhsT=expS[:, lcol:lcol + BS],
                                    rhs=V_all[:, h, c - 1, :],
                                    start=(j == 0), stop=(j == len(chs) - 1),
                                    skip_group_check=True,
                                )
                                if j == 0:  # snapshot local attention term
                                    nc.vector.reciprocal(rec[:, 1:2],
                                                         pv[:, 0:1])
                                    nc.vector.tensor_scalar_mul(
                                        t2, pv[:, 1:65], rec[:, 1:2])
                            if h == 0:
                                pvd = small.tile([128, 65], F32,
                                                 name="pvd", tag="pvd")
                                nc.vector.tensor_copy(out=pvd, in_=pv)
                                nc.sync.dma_start(
                                    dbg_pv[:, 65 * i:65 * (i + 1)], pvd)
                            nc.vector.reciprocal(rec[:, 0:1], pv[:, 0:1])
                            nc.vector.tensor_scalar_mul(t1, pv[:, 1:65],
                                                        rec[:, 0:1])
                            if i % 2 == 0:
                                an = small.tile([128, 128], BF16, name="an",
                                                tag="an")
                            nc.vector.tensor_tensor(
                                an[:, 64 * (i % 2):64 * (i % 2) + 64],
                                t1, t2, mybir.AluOpType.add)
                            if h == 0 and i % 2 == 1:
                                nc.sync.dma_start(
                                    dbg_an[:, 64 * (i - 1):64 * (i + 1)], an)
                            if i % 2 == 1:
                                tp = tpsum.tile([128, 128], BF16, name="tp",
                                                tag="tp")
                                nc.tensor.transpose(tp, an, ident)
                                nc.vector.tensor_copy(
                                    out=attnT[p0:p0 + 64, hp,
                                              BS * (i - 1):BS * i],
                                    in_=tp[0:64, :])
                                nc.vector.tensor_copy(
                                    out=attnT[p0:p0 + 64, hp,
                                              BS * i:BS * (i + 1)],
                                    in_=tp[64:128, :])

            nc.sync.dma_start(dbg_at[:, :, :], attnT)

            # ---------------- output projection ----------------
            with tc.tile_pool(name="opsum", bufs=3, space="PSUM") as opsum:
                wo_sb = wpool.tile([128, 8, E], BF16, name="wo_sb", tag="w")
                nc.sync.dma_start(wo_sb, wo_d)
                for sc in range(NBQ):
                    for fh in range(2):
                        psum = opsum.tile([128, 512], F32, name="opsum_t",
                                          tag="opsum")
                        for pr in range(NPAIR):
                            nc.tensor.matmul(
                                psum,
                                lhsT=attnT[:, pr, 128 * sc:128 * sc + 128],
                                rhs=wo_sb[:, pr, 512 * fh:512 * fh + 512],
                                start=(pr == 0), stop=(pr == NPAIR - 1),
                            )
                        osb = outp.tile([128, 512], F32, name="osb", tag="osb")
                        nc.vector.tensor_tensor(
                            osb, psum, bob2[:, 512 * fh:512 * fh + 512],
                            mybir.AluOpType.add)
                        nc.sync.dma_start(
                            out_t[128 * sc:128 * sc + 128,
                                  512 * fh:512 * fh + 512],
                            osb)

    nc.finalize()
    return nc


def TileCtx(nc):
    return tile.TileContext(nc)


_NC_CACHE = []


def _prep_inputs(q, k, v, Wq, bq, Wk, bk, Wv, bv, Wo, bo):
    bf = lambda x: np.ascontiguousarray(x).astype(_BF)
    wqT = bf(np.asarray(Wq, np.float32).T)
    wkT = bf(np.asarray(Wk, np.float32).T)
    wvT = bf(np.asarray(Wv, np.float32).T)
    woT = bf(np.asarray(Wo, np.float32).T)
    bq2 = np.ascontiguousarray(np.asarray(bq, np.float32).reshape(8, 128).T)
    bk2 = np.ascontiguousarray(np.asarray(bk, np.float32).reshape(8, 128).T)
    bvb = np.ascontiguousarray(
        np.broadcast_to(np.asarray(bv, np.float32), (128, E)))
    bob = np.ascontiguousarray(
        np.broadcast_to(np.asarray(bo, np.float32), (128, E)))

    in_maps = []
    for c in range(8):
        b, hf = c // 2, c % 2
        qb_ = np.asarray(q[b], np.float32).reshape(32, BS, E)
        kb_ = np.asarray(k[b], np.float32).reshape(32, BS, E)
        vb_ = np.asarray(v[b], np.float32).reshape(32, BS, E)
        if hf == 0:
            q_slab = qb_[0:16].reshape(SQ, E)
            k_slab = kb_[0:17].reshape(SKV, E)
            v_slab = vb_[0:17].reshape(SKV, E)
        else:
            q_slab = qb_[31:15:-1].reshape(SQ, E)
            k_slab = kb_[31:14:-1].reshape(SKV, E)
            v_slab = vb_[31:14:-1].reshape(SKV, E)
        in_maps.append({
            "qT": bf(q_slab.T), "kT": bf(k_slab.T), "vT": bf(v_slab.T),
            "wqT": wqT, "wkT": wkT, "wvT": wvT, "woT": woT,
            "bq2": bq2, "bk2": bk2, "bvb": bvb, "bob": bob,
        })
    return in_maps


def kernel(q, k, v, Wq, bq, Wk, bk, Wv, bv, Wo, bo, _trace=False):
    if not _NC_CACHE:
        _NC_CACHE.append(_build())
    nc = _NC_CACHE[0]
    in_maps = _prep_inputs(q, k, v, Wq, bq, Wk, bk, Wv, bv, Wo, bo)
    res = bass_utils.run_bass_kernel_spmd(
        nc, in_maps, core_ids=list(range(8)), trace=_trace)
    kernel.last_result = res
    out = np.empty((B, S, E), np.float32)
    for c in range(8):
        b, hf = c // 2, c % 2
        o = res.results[c]["out"]
        if hf == 0:
            out[b, 0:SQ] = o
        else:
            out[b, SQ:] = o.reshape(NBQ, BS, E)[::-1].reshape(SQ, E)
    return out


# revision 18
# speedup vs baseline: 1.0316x; 1.0316x over previous
"""BlockSparseAttention Trainium2 kernel (8 NeuronCores, SPMD).

Problem: B=4, S=4096, E=1024, H=16 heads, D=64, block=128, window = self +/- 1 block.
  Q/K/V projections -> block-local + windowed-cross attention -> output projection.

Sharding: core c = (batch b=c//2, seq half hf=c%2). Each core owns 16 q-blocks
(2048 rows) and a 17-block K/V slab (own 16 blocks + 1 halo block). The second
half is BLOCK-REVERSED on host so that the pad/halo structure is identical on
every core (uniform SPMD program):
  chunk 0 of the conceptual 18-chunk slab is always the invalid pad block and is
  simply skipped; shipped slab = chunks 1..17 (2176 rows).
  q-block i attends slab chunks {i, i+1, i+2}; self block = chunk i+1.

On-chip dataflow (all matmuls bf16 w/ fp32 PSUM accumulation):
  Q^T[e,s] = WqT-chunk.T @ qT      (lhsT=WqT [ein,eo], rhs=qT [ein,s])
  K^T[e,s] likewise -> bounced through DRAM, reloaded per head-pair
  V[s,e]   = vT-chunk.T @ WvT      (natural layout, + ones column for denoms)
  S^T[kk,q] = (K^T chunk).T @ Q^T  per (head, chunk) -> packed PSUM
  expS = exp(0.125 * S^T)          ACT, PSUM->SBUF bf16, packed [128, 6016]
  PV: out[q, 0:65] = expS-chunk.T @ [ones|V]  (col 0 = softmax denominator)
      cross accumulates 2-3 chunks; local = separate single-chunk group
  normalize per-partition (q on partitions), PE-transpose per block pair,
  collect attn^T[e, s] bf16, then O-projection + bias.
"""

import math
from contextlib import ExitStack

import numpy as np
import ml_dtypes

import concourse.bacc as bacc
import concourse.bass as bass
import concourse.mybir as mybir
import concourse.tile as tile
from concourse import bass_utils
from concourse.masks import make_identity

F32 = mybir.dt.float32
BF16 = mybir.dt.bfloat16

E = 1024
H = 16
D = 64
BS = 128
B = 4
S = 4096
NBQ = 16           # q blocks per core
SQ = NBQ * BS      # 2048
NCH = 17           # shipped kv chunks (c = 1..17)
SKV = NCH * BS     # 2176
NPAIR = 8          # head pairs
KCH = 8            # e_in chunks of 128
SCALE = 1.0 / math.sqrt(D)

_BF = ml_dtypes.bfloat16


def _chunk_qb(c):
    """Valid q-block range [qb0, qb1] attending slab chunk c (1..17)."""
    return max(0, c - 2), min(NBQ - 1, c)


def _score_layout():
    """Packed column layout of the per-head score matrix exp(S^T).

    Returns (chunk_base dict, total_cols). Chunk c occupies packed cols
    [chunk_base[c], chunk_base[c] + width_c) where width_c = 128 * n_valid_qblocks.
    """
    base = 0
    chunk_base = {}
    for c in range(1, NCH + 1):
        qb0, qb1 = _chunk_qb(c)
        chunk_base[c] = base
        base += (qb1 - qb0 + 1) * BS
    return chunk_base, base


CHUNK_BASE, SCORE_COLS = _score_layout()   # SCORE_COLS == 6016
SCORE_TILE = 1024                          # score psum tile width (2 banks)
N_SCORE_TILES = (SCORE_COLS + SCORE_TILE - 1) // SCORE_TILE


def _build():
    nc = bacc.Bacc(None, target_bir_lowering=False)

    qT_t = nc.dram_tensor("qT", [E, SQ], BF16, kind="ExternalInput")
    kT_t = nc.dram_tensor("kT", [E, SKV], BF16, kind="ExternalInput")
    vT_t = nc.dram_tensor("vT", [E, SKV], BF16, kind="ExternalInput")
    wqT_t = nc.dram_tensor("wqT", [E, E], BF16, kind="ExternalInput")
    wkT_t = nc.dram_tensor("wkT", [E, E], BF16, kind="ExternalInput")
    wvT_t = nc.dram_tensor("wvT", [E, E], BF16, kind="ExternalInput")
    woT_t = nc.dram_tensor("woT", [E, E], BF16, kind="ExternalInput")
    bq_t = nc.dram_tensor("bq2", [128, 8], F32, kind="ExternalInput")
    bk_t = nc.dram_tensor("bk2", [128, 8], F32, kind="ExternalInput")
    bvb_t = nc.dram_tensor("bvb", [128, E], F32, kind="ExternalInput")
    bob_t = nc.dram_tensor("bob", [128, E], F32, kind="ExternalInput")
    out_t = nc.dram_tensor("out", [SQ, E], F32, kind="ExternalOutput")



    qT_d = qT_t[:].rearrange("(a p) s -> p a s", p=128)       # [128, 8, 2048]
    kT_d = kT_t[:].rearrange("(a p) s -> p a s", p=128)       # [128, 8, 2176]
    vT_d = vT_t[:].rearrange("(a p) s -> p a s", p=128)
    wq_d = wqT_t[:].rearrange("(a p) f -> p a f", p=128)      # [128, 8, 1024]
    wk_d = wkT_t[:].rearrange("(a p) f -> p a f", p=128)
    wv_d = wvT_t[:].rearrange("(a p) f -> p a f", p=128)
    wo_d = woT_t[:].rearrange("(a p) f -> p a f", p=128)

    with TileCtx(nc) as tc:
        with (
            tc.tile_pool(name="dram", bufs=1, space="DRAM") as dpool,
            tc.tile_pool(name="const", bufs=1) as cpool,
            tc.tile_pool(name="big", bufs=1) as big,
            tc.tile_pool(name="wpool", bufs=1) as wpool,
            tc.tile_pool(name="stage", bufs=2) as stage,
            tc.tile_pool(name="ktpair", bufs=2) as ktpool,
            tc.tile_pool(name="exps", bufs=2) as epool,
            tc.tile_pool(name="small", bufs=4) as small,
            tc.tile_pool(name="outp", bufs=2) as outp,
        ):
            kt_dram = dpool.tile([E, SKV], BF16, name="kt_bounce")
            ktb_d = kt_dram.rearrange("(a p) s -> p a s", p=128)  # [128,8,2176]

            ident = cpool.tile([128, 128], BF16, name="ident")
            make_identity(nc, ident)
            bq_sb = cpool.tile([128, 8], F32, name="bq_sb")
            nc.sync.dma_start(bq_sb, bq_t[:, :])
            bk_sb = cpool.tile([128, 8], F32, name="bk_sb")
            nc.sync.dma_start(bk_sb, bk_t[:, :])
            bvb_sb = cpool.tile([128, E], F32, name="bvb_sb")
            nc.sync.dma_start(bvb_sb, bvb_t[:, :])
            bob_sb = cpool.tile([128, E], F32, name="bob_sb")
            nc.sync.dma_start(bob_sb, bob_t[:, :])

            # persistent big tensors
            QT_sb = big.tile([128, 8, SQ], BF16, name="QT_sb")       # Q^T
            V_all = big.tile([128, H, NCH, 65], BF16, name="V_all")  # [ones|V]
            V_flat = V_all.rearrange("p h c d -> p (h c d)")
            attnT = big.tile([128, 8, SQ], BF16, name="attnT")       # attn^T

            nc.gpsimd.memset(V_all[:, :, :, 0:1], 1.0)
            # DVE-local copy of the V bias so the V scatter copies depend on
            # DVE work only (instruction wait-slot limits on TensorTensor).
            bvb2 = cpool.tile([128, E], F32, name="bvb2")
            nc.vector.tensor_copy(out=bvb2, in_=bvb_sb)
            bob2 = cpool.tile([128, E], F32, name="bob2")
            nc.vector.tensor_copy(out=bob2, in_=bob_sb)
            bq2d = cpool.tile([128, 8], F32, name="bq2d")
            nc.vector.tensor_copy(out=bq2d, in_=bq_sb)
            bk2d = cpool.tile([128, 8], F32, name="bk2d")
            nc.vector.tensor_copy(out=bk2d, in_=bk_sb)

            # ---------------- projection phase ----------------
            with tc.tile_pool(name="ppsum", bufs=3, space="PSUM") as ppsum:
                # V projection: V[s, e_out] = vT_chunk.T @ WvT
                wv_sb = wpool.tile([128, 8, E], BF16, name="wv_sb", tag="w")
                nc.sync.dma_start(wv_sb, wv_d)
                for sg in range(5):                      # 4x512 + 1x128 cols of vT
                    w_sg = min(512, SKV - 512 * sg)
                    vstage = stage.tile([128, 8, 512], BF16, name="vstage",
                                        tag="xstage")
                    nc.sync.dma_start(vstage[:, :, :w_sg],
                                      vT_d[:, :, 512 * sg:512 * sg + w_sg])
                    for sc4 in range(w_sg // 128):       # s-chunks of 128
                        sc = 4 * sg + sc4
                        for eh in range(2):              # e_out halves of 512
                            psum = ppsum.tile([128, 512], F32, name="vpsum",
                                              tag="ppsum")
                            for kk in range(KCH):
                                nc.tensor.matmul(
                                    psum,
                                    lhsT=vstage[:, kk, 128 * sc4:128 * sc4 + 128],
                                    rhs=wv_sb[:, kk, 512 * eh:512 * eh + 512],
                                    start=(kk == 0), stop=(kk == KCH - 1),
                                )
                            # add bias, cast bf16, scatter to per-head layout
                            # (per-head 2D copies: a 3D+3D+3D tensor_tensor
                            # overflows the TT instruction's sync-wait slots)
                            for j in range(8):
                                h = 8 * eh + j
                                off = (h * NCH + sc) * 65 + 1
                                nc.vector.tensor_tensor(
                                    V_flat[:, off:off + 64],
                                    psum[:, 64 * j:64 * j + 64],
                                    bvb2[:, 64 * h:64 * h + 64],
                                    mybir.AluOpType.add,
                                )

                # Q projection: Q^T[e_out, s] = WqT_chunk.T @ qT
                wq_sb = wpool.tile([128, 8, E], BF16, name="wq_sb", tag="w")
                nc.sync.dma_start(wq_sb, wq_d)
                for st in range(4):                      # s tiles of 512
                    qstage = stage.tile([128, 8, 512], BF16, name="qstage",
                                        tag="xstage")
                    nc.sync.dma_start(qstage,
                                      qT_d[:, :, 512 * st:512 * st + 512])
                    for pr in range(NPAIR):
                        psum = ppsum.tile([128, 512], F32, name="qpsum",
                                          tag="ppsum")
                        for kk in range(KCH):
                            nc.tensor.matmul(
                                psum,
                                lhsT=wq_sb[:, kk, 128 * pr:128 * pr + 128],
                                rhs=qstage[:, kk, :],
                                start=(kk == 0), stop=(kk == KCH - 1),
                            )
                        nc.vector.tensor_scalar(
                            QT_sb[:, pr, 512 * st:512 * st + 512],
                            psum, bq2d[:, pr:pr + 1], None,
                            mybir.AluOpType.add,
                        )

                # K projection -> DRAM bounce (per head-pair rows)
                wk_sb = wpool.tile([128, 8, E], BF16, name="wk_sb", tag="w")
                nc.sync.dma_start(wk_sb, wk_d)
                for st in range(5):
                    w_st = min(512, SKV - 512 * st)
                    kstage = stage.tile([128, 8, 512], BF16, name="kstage",
                                        tag="xstage")
                    nc.sync.dma_start(kstage[:, :, :w_st],
                                      kT_d[:, :, 512 * st:512 * st + w_st])
                    for pr in range(NPAIR):
                        psum = ppsum.tile([128, 512], F32, name="kpsum",
                                          tag="ppsum")
                        for kk in range(KCH):
                            nc.tensor.matmul(
                                psum[:, :w_st],
                                lhsT=wk_sb[:, kk, 128 * pr:128 * pr + 128],
                                rhs=kstage[:, kk, :w_st],
                                start=(kk == 0), stop=(kk == KCH - 1),
                            )
                        ktmp = small.tile([128, 512], BF16, name="ktmp",
                                          tag="ktmp")
                        nc.vector.tensor_scalar(
                            ktmp[:, :w_st], psum[:, :w_st],
                            bk2d[:, pr:pr + 1], None, mybir.AluOpType.add,
                        )
                        nc.sync.dma_start(
                            ktb_d[:, pr, 512 * st:512 * st + w_st],
                            ktmp[:, :w_st])

            # ---------------- attention phase ----------------
            with (
                tc.tile_pool(name="spsum", bufs=2, space="PSUM") as spsum,
                tc.tile_pool(name="pvpsum", bufs=3, space="PSUM") as pvpsum,
                tc.tile_pool(name="tpsum", bufs=1, space="PSUM") as tpsum,
            ):
                for hp in range(NPAIR):
                    kt_pair = ktpool.tile([128, SKV], BF16, name="kt_pair",
                                          tag="ktpair")
                    nc.sync.dma_start(kt_pair, ktb_d[:, hp, :])
                    for hh in range(2):
                        h = 2 * hp + hh
                        p0 = 64 * hh
                        expS = epool.tile([128, SCORE_COLS], BF16,
                                          name="expS", tag="expS")

                        # scores S^T, packed into [128, 1024] psum tiles
                        score_ps = {}
                        for c in range(1, NCH + 1):
                            qb0, qb1 = _chunk_qb(c)
                            cb = CHUNK_BASE[c]
                            width = (qb1 - qb0 + 1) * BS
                            pos = cb
                            while pos < cb + width:
                                # split at 512-boundaries of packed layout
                                nxt = min(cb + width, (pos // 512 + 1) * 512)
                                t = pos // SCORE_TILE
                                if t not in score_ps:
                                    score_ps[t] = spsum.tile(
                                        [128, SCORE_TILE], F32,
                                        name="score_ps", tag="spsum")
                                qcol = qb0 * BS + (pos - cb)
                                nc.tensor.matmul(
                                    score_ps[t][:, pos - SCORE_TILE * t:
                                                nxt - SCORE_TILE * t],
                                    lhsT=kt_pair[p0:p0 + 64,
                                                 128 * (c - 1):128 * c],
                                    rhs=QT_sb[p0:p0 + 64, hp,
                                              qcol:qcol + (nxt - pos)],
                                    start=True, stop=True,
                                )
                                pos = nxt
                                # exp as soon as a tile is complete
                                done = (c == NCH and pos == cb + width)
                                if pos % SCORE_TILE == 0 or done:
                                    tt = (pos - 1) // SCORE_TILE
                                    if tt in score_ps:
                                        wt = min(SCORE_TILE,
                                                 SCORE_COLS - SCORE_TILE * tt)
                                        nc.scalar.activation(
                                            expS[:, SCORE_TILE * tt:
                                                 SCORE_TILE * tt + wt],
                                            score_ps[tt][:, :wt],
                                            mybir.ActivationFunctionType.Exp,
                                            scale=SCALE,
                                        )
                                        del score_ps[tt]

                        # PV + normalize + transpose, per q-block.
                        # Single accumulation group per PSUM bank: the self
                        # chunk goes first and its partial state (= the local
                        # attention term) is snapshotted by DVE mid-group.
                        an = None
                        for i in range(NBQ):
                            chs = [i + 1] + [c for c in (i, i + 2) if c >= 1]
                            pv = pvpsum.tile([128, 65], F32, name="pv",
                                             tag="pv")
                            rec = small.tile([128, 2], F32, name="rec",
                                             tag="rec")
                            t1 = small.tile([128, 64], F32, name="t1", tag="t1")
                            t2 = small.tile([128, 64], F32, name="t2", tag="t2")
                            for j, c in enumerate(chs):
                                qb0, _ = _chunk_qb(c)
                                lcol = CHUNK_BASE[c] + (i - qb0) * BS
                                nc.tensor.matmul(
                                    pv,
                                    lhsT=expS[:, lcol:lcol + BS],
                                    rhs=V_all[:, h, c - 1, :],
                                    start=(j == 0), stop=(j == len(chs) - 1),
                                    skip_group_check=True,
                                )
                                if j == 0:  # snapshot local attention term
                                    nc.vector.reciprocal(rec[:, 1:2],
                                                         pv[:, 0:1])
                                    nc.vector.tensor_scalar_mul(
                                        t2, pv[:, 1:65], rec[:, 1:2])
                            nc.vector.reciprocal(rec[:, 0:1], pv[:, 0:1])
                            nc.vector.tensor_scalar_mul(t1, pv[:, 1:65],
                                                        rec[:, 0:1])
                            if i % 2 == 0:
                                an = small.tile([128, 128], BF16, name="an",
                                                tag="an")
                            nc.vector.tensor_tensor(
                                an[:, 64 * (i % 2):64 * (i % 2) + 64],
                                t1, t2, mybir.AluOpType.add)
                            if i % 2 == 1:
                                tp = tpsum.tile([128, 128], BF16, name="tp",
                                                tag="tp")
                                nc.tensor.transpose(tp, an, ident)
                                nc.vector.tensor_copy(
                                    out=attnT[p0:p0 + 64, hp,
                                              BS * (i - 1):BS * i],
                                    in_=tp[0:64, :])
                                nc.vector.tensor_copy(
                                    out=attnT[p0:p0 + 64, hp,
                                              BS * i:BS * (i + 1)],
                                    in_=tp[64:128, :])

            # ---------------- output projection ----------------
            with tc.tile_pool(name="opsum", bufs=3, space="PSUM") as opsum:
                wo_sb = wpool.tile([128, 8, E], BF16, name="wo_sb", tag="w")
                nc.sync.dma_start(wo_sb, wo_d)
                for sc in range(NBQ):
                    for fh in range(2):
                        psum = opsum.tile([128, 512], F32, name="opsum_t",
                                          tag="opsum")
                        for pr in range(NPAIR):
                            nc.tensor.matmul(
                                psum,
                                lhsT=attnT[:, pr, 128 * sc:128 * sc + 128],
                                rhs=wo_sb[:, pr, 512 * fh:512 * fh + 512],
                                start=(pr == 0), stop=(pr == NPAIR - 1),
                            )
                        osb = outp.tile([128, 512], F32, name="osb", tag="osb")
                        nc.vector.tensor_tensor(
                            osb, psum, bob2[:, 512 * fh:512 * fh + 512],
                            mybir.AluOpType.add)
                        nc.sync.dma_start(
                            out_t[128 * sc:128 * sc + 128,
                                  512 * fh:512 * fh + 512],
                            osb)

    nc.finalize()
    return nc


def TileCtx(nc):
    return tile.TileContext(nc)


_NC_CACHE = []


def _prep_inputs(q, k, v, Wq, bq, Wk, bk, Wv, bv, Wo, bo):
    bf = lambda x: np.ascontiguousarray(x).astype(_BF)
    wqT = bf(np.asarray(Wq, np.float32).T)
    wkT = bf(np.asarray(Wk, np.float32).T)
    wvT = bf(np.asarray(Wv, np.float32).T)
    woT = bf(np.asarray(Wo, np.float32).T)
    bq2 = np.ascontiguousarray(np.asarray(bq, np.float32).reshape(8, 128).T)
    bk2 = np.ascontiguousarray(np.asarray(bk, np.float32).reshape(8, 128).T)
    bvb = np.ascontiguousarray(
        np.broadcast_to(np.asarray(bv, np.float32), (128, E)))
    bob = np.ascontiguousarray(
        np.broadcast_to(np.asarray(bo, np.float32), (128, E)))

    in_maps = []
    for c in range(8):
        b, hf = c // 2, c % 2
        qb_ = np.asarray(q[b], np.float32).reshape(32, BS, E)
        kb_ = np.asarray(k[b], np.float32).reshape(32, BS, E)
        vb_ = np.asarray(v[b], np.float32).reshape(32, BS, E)
        if hf == 0:
            q_slab = qb_[0:16].reshape(SQ, E)
            k_slab = kb_[0:17].reshape(SKV, E)
            v_slab = vb_[0:17].reshape(SKV, E)
        else:
            q_slab = qb_[31:15:-1].reshape(SQ, E)
            k_slab = kb_[31:14:-1].reshape(SKV, E)
            v_slab = vb_[31:14:-1].reshape(SKV, E)
        in_maps.append({
            "qT": bf(q_slab.T), "kT": bf(k_slab.T), "vT": bf(v_slab.T),
            "wqT": wqT, "wkT": wkT, "wvT": wvT, "woT": woT,
            "bq2": bq2, "bk2": bk2, "bvb": bvb, "bob": bob,
        })
    return in_maps


def kernel(q, k, v, Wq, bq, Wk, bk, Wv, bv, Wo, bo, _trace=False):
    if not _NC_CACHE:
        _NC_CACHE.append(_build())
    nc = _NC_CACHE[0]
    in_maps = _prep_inputs(q, k, v, Wq, bq, Wk, bk, Wv, bv, Wo, bo)
    res = bass_utils.run_bass_kernel_spmd(
        nc, in_maps, core_ids=list(range(8)), trace=_trace)
    kernel.last_result = res
    out = np.empty((B, S, E), np.float32)
    for c in range(8):
        b, hf = c // 2, c % 2
        o = res.results[c]["out"]
        if hf == 0:
            out[b, 0:SQ] = o
        else:
            out[b, SQ:] = o.reshape(NBQ, BS, E)[::-1].reshape(SQ, E)
    return out


# revision 20
# speedup vs baseline: 1.3199x; 1.2794x over previous
"""BlockSparseAttention Trainium2 kernel (8 NeuronCores, SPMD).

Problem: B=4, S=4096, E=1024, H=16 heads, D=64, block=128, window = self +/- 1 block.
  Q/K/V projections -> block-local + windowed-cross attention -> output projection.

Sharding: core c = (batch b=c//2, seq half hf=c%2). Each core owns 16 q-blocks
(2048 rows) and a 17-block K/V slab (own 16 blocks + 1 halo block). The second
half is BLOCK-REVERSED on host so that the pad/halo structure is identical on
every core (uniform SPMD program):
  chunk 0 of the conceptual 18-chunk slab is always the invalid pad block and is
  simply skipped; shipped slab = chunks 1..17 (2176 rows).
  q-block i attends slab chunks {i, i+1, i+2}; self block = chunk i+1.

On-chip dataflow (all matmuls bf16 w/ fp32 PSUM accumulation):
  Q^T[e,s] = WqT-chunk.T @ qT      (lhsT=WqT [ein,eo], rhs=qT [ein,s])
  K^T[e,s] likewise -> bounced through DRAM, reloaded per head-pair
  V[s,e]   = vT-chunk.T @ WvT      (natural layout, + ones column for denoms)
  S^T[kk,q] = (K^T chunk).T @ Q^T  per (head, chunk) -> packed PSUM
  expS = exp(0.125 * S^T)          ACT, PSUM->SBUF bf16, packed [128, 6016]
  PV: out[q, 0:65] = expS-chunk.T @ [ones|V]  (col 0 = softmax denominator)
      cross accumulates 2-3 chunks; local = separate single-chunk group
  normalize per-partition (q on partitions), PE-transpose per block pair,
  collect attn^T[e, s] bf16, then O-projection + bias.
"""

import math
from contextlib import ExitStack

import numpy as np
import ml_dtypes

import concourse.bacc as bacc
import concourse.bass as bass
import concourse.mybir as mybir
import concourse.tile as tile
from concourse import bass_utils
from concourse.masks import make_identity

F32 = mybir.dt.float32
BF16 = mybir.dt.bfloat16

E = 1024
H = 16
D = 64
BS = 128
B = 4
S = 4096
NBQ = 16           # q blocks per core
SQ = NBQ * BS      # 2048
NCH = 17           # shipped kv chunks (c = 1..17)
SKV = NCH * BS     # 2176
NPAIR = 8          # head pairs
KCH = 8            # e_in chunks of 128
SCALE = 1.0 / math.sqrt(D)

_BF = ml_dtypes.bfloat16


def _chunk_qb(c):
    """Valid q-block range [qb0, qb1] attending slab chunk c (1..17)."""
    return max(0, c - 2), min(NBQ - 1, c)


def _score_layout():
    """Packed column layout of the per-head score matrix exp(S^T).

    Returns (chunk_base dict, total_cols). Chunk c occupies packed cols
    [chunk_base[c], chunk_base[c] + width_c) where width_c = 128 * n_valid_qblocks.
    """
    base = 0
    chunk_base = {}
    for c in range(1, NCH + 1):
        qb0, qb1 = _chunk_qb(c)
        chunk_base[c] = base
        base += (qb1 - qb0 + 1) * BS
    return chunk_base, base


CHUNK_BASE, SCORE_COLS = _score_layout()   # SCORE_COLS == 6016
SCORE_TILE = 1024                          # score psum tile width (2 banks)
N_SCORE_TILES = (SCORE_COLS + SCORE_TILE - 1) // SCORE_TILE


def _build():
    nc = bacc.Bacc(None, target_bir_lowering=False)

    qT_t = nc.dram_tensor("qT", [E, SQ], BF16, kind="ExternalInput")
    kT_t = nc.dram_tensor("kT", [E, SKV], BF16, kind="ExternalInput")
    vT_t = nc.dram_tensor("vT", [E, SKV], BF16, kind="ExternalInput")
    wqT_t = nc.dram_tensor("wqT", [E, E], BF16, kind="ExternalInput")
    wkT_t = nc.dram_tensor("wkT", [E, E], BF16, kind="ExternalInput")
    wvT_t = nc.dram_tensor("wvT", [E, E], BF16, kind="ExternalInput")
    woT_t = nc.dram_tensor("woT", [E, E], BF16, kind="ExternalInput")
    bq_t = nc.dram_tensor("bq2", [128, 8], F32, kind="ExternalInput")
    bk_t = nc.dram_tensor("bk2", [128, 8], F32, kind="ExternalInput")
    bvb_t = nc.dram_tensor("bvb", [128, E], F32, kind="ExternalInput")
    bob_t = nc.dram_tensor("bob", [128, E], F32, kind="ExternalInput")
    out_t = nc.dram_tensor("out", [SQ, E], F32, kind="ExternalOutput")



    qT_d = qT_t[:].rearrange("(a p) s -> p a s", p=128)       # [128, 8, 2048]
    kT_d = kT_t[:].rearrange("(a p) s -> p a s", p=128)       # [128, 8, 2176]
    vT_d = vT_t[:].rearrange("(a p) s -> p a s", p=128)
    wq_d = wqT_t[:].rearrange("(a p) f -> p a f", p=128)      # [128, 8, 1024]
    wk_d = wkT_t[:].rearrange("(a p) f -> p a f", p=128)
    wv_d = wvT_t[:].rearrange("(a p) f -> p a f", p=128)
    wo_d = woT_t[:].rearrange("(a p) f -> p a f", p=128)

    with TileCtx(nc) as tc:
        with (
            tc.tile_pool(name="dram", bufs=1, space="DRAM") as dpool,
            tc.tile_pool(name="const", bufs=1) as cpool,
            tc.tile_pool(name="big", bufs=1) as big,
            tc.tile_pool(name="wpool", bufs=1) as wpool,
            tc.tile_pool(name="stage", bufs=2) as stage,
            tc.tile_pool(name="ktpair", bufs=2) as ktpool,
            tc.tile_pool(name="exps", bufs=2) as epool,
            tc.tile_pool(name="small", bufs=4) as small,
            tc.tile_pool(name="outp", bufs=2) as outp,
        ):
            kt_dram = dpool.tile([E, SKV], BF16, name="kt_bounce")
            ktb_d = kt_dram.rearrange("(a p) s -> p a s", p=128)  # [128,8,2176]
            qt_dram = dpool.tile([E, SQ], BF16, name="qt_bounce")
            qtb_d = qt_dram.rearrange("(a p) s -> p a s", p=128)  # [128,8,2048]

            ident = cpool.tile([128, 128], BF16, name="ident")
            make_identity(nc, ident)
            bq_sb = cpool.tile([128, 8], F32, name="bq_sb")
            nc.sync.dma_start(bq_sb, bq_t[:, :])
            bk_sb = cpool.tile([128, 8], F32, name="bk_sb")
            nc.sync.dma_start(bk_sb, bk_t[:, :])
            bvb_sb = cpool.tile([128, E], F32, name="bvb_sb")
            nc.sync.dma_start(bvb_sb, bvb_t[:, :])
            bob_sb = cpool.tile([128, E], F32, name="bob_sb")
            nc.sync.dma_start(bob_sb, bob_t[:, :])

            # persistent big tensors
            V_all = big.tile([128, H, NCH, 65], BF16, name="V_all")  # [ones|V]
            V_flat = V_all.rearrange("p h c d -> p (h c d)")
            attnT = big.tile([128, 8, SQ], BF16, name="attnT")       # attn^T

            nc.gpsimd.memset(V_all[:, :, :, 0:1], 1.0)

            # ---------------- projection phase ----------------
            with tc.tile_pool(name="ppsum", bufs=3, space="PSUM") as ppsum:
                # V projection: V[s, e_out] = vT_chunk.T @ WvT
                wv_sb = wpool.tile([128, 8, E], BF16, name="wv_sb", tag="w")
                nc.sync.dma_start(wv_sb, wv_d)
                for sg in range(5):                      # 4x512 + 1x128 cols of vT
                    w_sg = min(512, SKV - 512 * sg)
                    vstage = stage.tile([128, 8, 512], BF16, name="vstage",
                                        tag="xstage")
                    nc.sync.dma_start(vstage[:, :, :w_sg],
                                      vT_d[:, :, 512 * sg:512 * sg + w_sg])
                    for sc4 in range(w_sg // 128):       # s-chunks of 128
                        sc = 4 * sg + sc4
                        for eh in range(2):              # e_out halves of 512
                            psum = ppsum.tile([128, 512], F32, name="vpsum",
                                              tag="ppsum")
                            for kk in range(KCH):
                                nc.tensor.matmul(
                                    psum,
                                    lhsT=vstage[:, kk, 128 * sc4:128 * sc4 + 128],
                                    rhs=wv_sb[:, kk, 512 * eh:512 * eh + 512],
                                    start=(kk == 0), stop=(kk == KCH - 1),
                                )
                            # add bias, cast bf16, scatter to per-head layout
                            # (per-head 2D copies: a 3D+3D+3D tensor_tensor
                            # overflows the TT instruction's sync-wait slots)
                            for j in range(8):
                                h = 8 * eh + j
                                off = (h * NCH + sc) * 65 + 1
                                nc.vector.tensor_tensor(
                                    V_flat[:, off:off + 64],
                                    psum[:, 64 * j:64 * j + 64],
                                    bvb_sb[:, 64 * h:64 * h + 64],
                                    mybir.AluOpType.add,
                                )

                # Q projection: Q^T[e_out, s] = WqT_chunk.T @ qT
                wq_sb = wpool.tile([128, 8, E], BF16, name="wq_sb", tag="w")
                nc.sync.dma_start(wq_sb, wq_d)
                for st in range(4):                      # s tiles of 512
                    qstage = stage.tile([128, 8, 512], BF16, name="qstage",
                                        tag="xstage")
                    nc.sync.dma_start(qstage,
                                      qT_d[:, :, 512 * st:512 * st + 512])
                    for pr in range(NPAIR):
                        psum = ppsum.tile([128, 512], F32, name="qpsum",
                                          tag="ppsum")
                        for kk in range(KCH):
                            nc.tensor.matmul(
                                psum,
                                lhsT=wq_sb[:, kk, 128 * pr:128 * pr + 128],
                                rhs=qstage[:, kk, :],
                                start=(kk == 0), stop=(kk == KCH - 1),
                            )
                        qtmp = small.tile([128, 512], BF16, name="qtmp",
                                          tag="ktmp", bufs=2)
                        nc.vector.tensor_scalar(
                            qtmp, psum, bq_sb[:, pr:pr + 1], None,
                            mybir.AluOpType.add,
                        )
                        nc.sync.dma_start(
                            qtb_d[:, pr, 512 * st:512 * st + 512], qtmp)

                # K projection -> DRAM bounce (per head-pair rows)
                wk_sb = wpool.tile([128, 8, E], BF16, name="wk_sb", tag="w")
                nc.sync.dma_start(wk_sb, wk_d)
                for st in range(5):
                    w_st = min(512, SKV - 512 * st)
                    kstage = stage.tile([128, 8, 512], BF16, name="kstage",
                                        tag="xstage")
                    nc.sync.dma_start(kstage[:, :, :w_st],
                                      kT_d[:, :, 512 * st:512 * st + w_st])
                    for pr in range(NPAIR):
                        psum = ppsum.tile([128, 512], F32, name="kpsum",
                                          tag="ppsum")
                        for kk in range(KCH):
                            nc.tensor.matmul(
                                psum[:, :w_st],
                                lhsT=wk_sb[:, kk, 128 * pr:128 * pr + 128],
                                rhs=kstage[:, kk, :w_st],
                                start=(kk == 0), stop=(kk == KCH - 1),
                            )
                        ktmp = small.tile([128, 512], BF16, name="ktmp",
                                          tag="ktmp", bufs=2)
                        nc.vector.tensor_scalar(
                            ktmp[:, :w_st], psum[:, :w_st],
                            bk_sb[:, pr:pr + 1], None, mybir.AluOpType.add,
                        )
                        nc.sync.dma_start(
                            ktb_d[:, pr, 512 * st:512 * st + w_st],
                            ktmp[:, :w_st])

            # ---------------- attention phase ----------------
            # Per head-pair: the two heads' score matmuls are emitted
            # interleaved so their 64-partition matmuls land in disjoint PE
            # row groups (0-63 / 64-127) and execute concurrently.
            # PV: packed PSUM banks, 4 q-blocks per bank, cross and local
            # terms in separate banks (only sequential accumulation groups
            # within a bank -- interleaved groups clobber has_written).
            with (
                tc.tile_pool(name="spsum", bufs=2, space="PSUM") as spsum,
                tc.tile_pool(name="pvcp", bufs=2, space="PSUM") as pvcp,
                tc.tile_pool(name="pvlp", bufs=1, space="PSUM") as pvlp,
                tc.tile_pool(name="tpsum", bufs=1, space="PSUM") as tpsum,
            ):
                for hp in range(NPAIR):
                    kt_pair = ktpool.tile([128, SKV], BF16, name="kt_pair",
                                          tag="ktpair")
                    nc.sync.dma_start(kt_pair, ktb_d[:, hp, :])
                    qt_pair = ktpool.tile([128, SQ], BF16, name="qt_pair",
                                          tag="qtpair")
                    nc.sync.dma_start(qt_pair, qtb_d[:, hp, :])
                    expS2 = [
                        epool.tile([128, SCORE_COLS], BF16,
                                   name=f"expS{hh}", tag=f"expS{hh}")
                        for hh in range(2)
                    ]

                    # scores S^T for both heads, interleaved by row group
                    score_ps = {}
                    for c in range(1, NCH + 1):
                        qb0, qb1 = _chunk_qb(c)
                        cb = CHUNK_BASE[c]
                        width = (qb1 - qb0 + 1) * BS
                        pos = cb
                        while pos < cb + width:
                            nxt = min(cb + width, (pos // 512 + 1) * 512)
                            t = pos // SCORE_TILE
                            qcol = qb0 * BS + (pos - cb)
                            for hh in range(2):
                                if (hh, t) not in score_ps:
                                    score_ps[(hh, t)] = spsum.tile(
                                        [128, SCORE_TILE], F32,
                                        name="score_ps", tag="spsum")
                                p0 = 64 * hh
                                nc.tensor.matmul(
                                    score_ps[(hh, t)][:, pos - SCORE_TILE * t:
                                                      nxt - SCORE_TILE * t],
                                    lhsT=kt_pair[p0:p0 + 64,
                                                 128 * (c - 1):128 * c],
                                    rhs=qt_pair[p0:p0 + 64,
                                                qcol:qcol + (nxt - pos)],
                                    start=True, stop=True,
                                )
                            pos = nxt
                            done = (c == NCH and pos == cb + width)
                            if pos % SCORE_TILE == 0 or done:
                                tt = (pos - 1) // SCORE_TILE
                                for hh in range(2):
                                    if (hh, tt) in score_ps:
                                        wt = min(SCORE_TILE,
                                                 SCORE_COLS - SCORE_TILE * tt)
                                        nc.scalar.activation(
                                            expS2[hh][:, SCORE_TILE * tt:
                                                      SCORE_TILE * tt + wt],
                                            score_ps[(hh, tt)][:, :wt],
                                            mybir.ActivationFunctionType.Exp,
                                            scale=SCALE,
                                        )
                                        del score_ps[(hh, tt)]

                    # PV + batched normalize + transpose, 4 q-blocks per group
                    for hh in range(2):
                        h = 2 * hp + hh
                        p0 = 64 * hh
                        expS = expS2[hh]
                        for g in range(NBQ // 4):
                            pvc = pvcp.tile([128, 260], F32, name="pvc",
                                            tag="pvc")
                            pvl = pvlp.tile([128, 260], F32, name="pvl",
                                            tag="pvl")
                            for b in range(4):
                                i = 4 * g + b
                                chs = [c for c in (i, i + 1, i + 2) if c >= 1]
                                for j, c in enumerate(chs):
                                    qb0, _ = _chunk_qb(c)
                                    lcol = CHUNK_BASE[c] + (i - qb0) * BS
                                    nc.tensor.matmul(
                                        pvc[:, 65 * b:65 * b + 65],
                                        lhsT=expS[:, lcol:lcol + BS],
                                        rhs=V_all[:, h, c - 1, :],
                                        start=(j == 0),
                                        stop=(j == len(chs) - 1),
                                        skip_group_check=True,
                                    )
                                    if c == i + 1:
                                        # local term, same stationary operand
                                        nc.tensor.matmul(
                                            pvl[:, 65 * b:65 * b + 65],
                                            lhsT=expS[:, lcol:lcol + BS],
                                            rhs=V_all[:, h, c - 1, :],
                                            start=True, stop=True,
                                            skip_group_check=True,
                                        )
                            rec = small.tile([128, 8], F32, name="rec",
                                             tag="rec")
                            nc.vector.reciprocal(rec[:, 0:4],
                                                 pvc[:, 0:260:65])
                            nc.vector.reciprocal(rec[:, 4:8],
                                                 pvl[:, 0:260:65])
                            t1 = small.tile([128, 256], F32, name="t1",
                                            tag="t1", bufs=2)
                            t2 = small.tile([128, 256], F32, name="t2",
                                            tag="t2", bufs=2)
                            pvc4 = pvc.rearrange("p (b x) -> p b x", x=65)
                            pvl4 = pvl.rearrange("p (b x) -> p b x", x=65)
                            t1_3 = t1.rearrange("p (b x) -> p b x", x=64)
                            t2_3 = t2.rearrange("p (b x) -> p b x", x=64)
                            nc.vector.tensor_tensor(
                                t1_3, pvc4[:, :, 1:65],
                                rec[:, 0:4, None].to_broadcast([128, 4, 64]),
                                mybir.AluOpType.mult)
                            nc.vector.tensor_tensor(
                                t2_3, pvl4[:, :, 1:65],
                                rec[:, 4:8, None].to_broadcast([128, 4, 64]),
                                mybir.AluOpType.mult)
                            an4 = small.tile([128, 256], BF16, name="an4",
                                             tag="an4", bufs=2)
                            nc.vector.tensor_tensor(an4, t1, t2,
                                                    mybir.AluOpType.add)
                            tp4 = tpsum.tile([64, 512], BF16, name="tp4",
                                             tag="tp4")
                            for b in range(4):
                                nc.tensor.transpose(
                                    tp4[:, 128 * b:128 * b + 128],
                                    an4[:, 64 * b:64 * b + 64], ident)
                            nc.vector.tensor_copy(
                                out=attnT[p0:p0 + 64, hp,
                                          512 * g:512 * g + 512],
                                in_=tp4)

            # ---------------- output projection ----------------
            with tc.tile_pool(name="opsum", bufs=3, space="PSUM") as opsum:
                wo_sb = wpool.tile([128, 8, E], BF16, name="wo_sb", tag="w")
                nc.sync.dma_start(wo_sb, wo_d)
                for sc in range(NBQ):
                    for fh in range(2):
                        psum = opsum.tile([128, 512], F32, name="opsum_t",
                                          tag="opsum")
                        for pr in range(NPAIR):
                            nc.tensor.matmul(
                                psum,
                                lhsT=attnT[:, pr, 128 * sc:128 * sc + 128],
                                rhs=wo_sb[:, pr, 512 * fh:512 * fh + 512],
                                start=(pr == 0), stop=(pr == NPAIR - 1),
                            )
                        osb = outp.tile([128, 512], F32, name="osb", tag="osb")
                        nc.vector.tensor_tensor(
                            osb, psum, bob_sb[:, 512 * fh:512 * fh + 512],
                            mybir.AluOpType.add)
                        nc.sync.dma_start(
                            out_t[128 * sc:128 * sc + 128,
                                  512 * fh:512 * fh + 512],
                            osb)

    nc.finalize()
    return nc


def TileCtx(nc):
    return tile.TileContext(nc)


_NC_CACHE = []


def _prep_inputs(q, k, v, Wq, bq, Wk, bk, Wv, bv, Wo, bo):
    bf = lambda x: np.ascontiguousarray(x).astype(_BF)
    wqT = bf(np.asarray(Wq, np.float32).T)
    wkT = bf(np.asarray(Wk, np.float32).T)
    wvT = bf(np.asarray(Wv, np.float32).T)
    woT = bf(np.asarray(Wo, np.float32).T)
    bq2 = np.ascontiguousarray(np.asarray(bq, np.float32).reshape(8, 128).T)
    bk2 = np.ascontiguousarray(np.asarray(bk, np.float32).reshape(8, 128).T)
    bvb = np.ascontiguousarray(
        np.broadcast_to(np.asarray(bv, np.float32), (128, E)))
    bob = np.ascontiguousarray(
        np.broadcast_to(np.asarray(bo, np.float32), (128, E)))

    in_maps = []
    for c in range(8):
        b, hf = c // 2, c % 2
        qb_ = np.asarray(q[b], np.float32).reshape(32, BS, E)
        kb_ = np.asarray(k[b], np.float32).reshape(32, BS, E)
        vb_ = np.asarray(v[b], np.float32).reshape(32, BS, E)
        if hf == 0:
            q_slab = qb_[0:16].reshape(SQ, E)
            k_slab = kb_[0:17].reshape(SKV, E)
            v_slab = vb_[0:17].reshape(SKV, E)
        else:
            q_slab = qb_[31:15:-1].reshape(SQ, E)
            k_slab = kb_[31:14:-1].reshape(SKV, E)
            v_slab = vb_[31:14:-1].reshape(SKV, E)
        in_maps.append({
            "qT": bf(q_slab.T), "kT": bf(k_slab.T), "vT": bf(v_slab.T),
            "wqT": wqT, "wkT": wkT, "wvT": wvT, "woT": woT,
            "bq2": bq2, "bk2": bk2, "bvb": bvb, "bob": bob,
        })
    return in_maps


def kernel(q, k, v, Wq, bq, Wk, bk, Wv, bv, Wo, bo, _trace=False):
    if not _NC_CACHE:
        _NC_CACHE.append(_build())
    nc = _NC_CACHE[0]
    in_maps = _prep_inputs(q, k, v, Wq, bq, Wk, bk, Wv, bv, Wo, bo)
    res = bass_utils.run_bass_kernel_spmd(
        nc, in_maps, core_ids=list(range(8)), trace=_trace)
    kernel.last_result = res
    out = np.empty((B, S, E), np.float32)
    for c in range(8):
        b, hf = c // 2, c % 2
        o = res.results[c]["out"]
        if hf == 0:
            out[b, 0:SQ] = o
        else:
            out[b, SQ:] = o.reshape(NBQ, BS, E)[::-1].reshape(SQ, E)
    return out
